# revision 10
# baseline (speedup 1.0000x reference)
"""Trainium2 Bass kernel for nn_Attention_40991167873617 (sparse_attention).

Computation (reference):
    ep    = x[:,0] * x[:,1]                          # [B, E]
    trees = x[:,2:]                                  # [B, T, E]
    h     = relu(cat([ep, trees], -1) @ attn_w + b)  # [B, T, A]
    l     = h @ proj_w (+ proj_b)                    # [B, T, 1]
    s     = softmax(l, axis=1)
    out   = sum(s * trees, 1) / T                    # [B, E]
    returns (out, ep)

Strategy:
  - Pure data-parallel over 8 cores (B/8 = 1024 rows each); weights replicated.
  - Host uploads trees TRANSPOSED ([E, B_c*T]) so the E-contraction matmul
    streams with perfect DMA and no on-chip transpose of the big tensor.
  - Main matmul in transposed orientation: hT[A, rows] = W2T.T @ treesT,
    with the per-batch ep@W1 term folded in via a K=8 one-hot matmul and
    attn_b folded into the relu's per-partition bias.
  - proj_b dropped (softmax is shift invariant).
  - logits via K=A matmuls -> [1, rows]; chunk-wise SBUF->SBUF DMA reshapes
    to [64, T] for lane-parallel softmax; exp(x - max) on ACT.
  - weighted tree sum: broadcast w to 128 partitions via a K=1 ones matmul,
    DVE multiply with treesT, segmented (per-b) reduce over t on GPSIMD.
  - final [E, b] -> [b, E] via small PE transposes; scale by 1/(T*Z).
"""

import sys

sys.path.insert(0, "/opt/trn_rl_repo")

from contextlib import ExitStack

import numpy as np

import concourse.bacc as bacc
import concourse.tile as tile
from concourse import mybir
from concourse.alu_op_type import AluOpType
from concourse.bass_utils import run_bass_kernel_spmd

AF = mybir.ActivationFunctionType
AX = mybir.AxisListType
F32 = mybir.dt.float32
F32R = mybir.dt.float32r

B, T, E, A = 8192, 64, 256, 256
NCORES = 8
BC = B // NCORES          # 1024 batch rows per core
ROWS = BC * T             # 65536 (b, t) rows per core
RB = 512                  # rows per block
BPB = RB // T             # 8 batch rows per block
NBLK = ROWS // RB         # 128 blocks per core
CHUNK_BLKS = 4            # blocks per chunk (softmax granularity)
NCHUNK = NBLK // CHUNK_BLKS
CB = CHUNK_BLKS * BPB     # 64 batch rows per chunk
CROWS = CHUNK_BLKS * RB   # 4096 rows per chunk

USE_F32R = True

PROFILE = False
LAST_EXEC_NS = None
LAST_RESULTS = None

_CACHE = {}


DT_R = F32R if USE_F32R else F32


def _mmt(ap):
    return ap


def _f32(ap):
    return ap.bitcast(F32) if USE_F32R else ap


def _body(ctx, tc, ins, outs):
    nc = tc.nc
    tT_d, x01_d, aw_d, ab_d, pw_d, oh_d, on_d, id_d = ins
    oa_d, oe_d = outs

    consts = ctx.enter_context(tc.tile_pool(name="consts", bufs=1))

    # --- load constants -------------------------------------------------
    wsb = consts.tile([128, 4 * A], DT_R, tag="wsb")       # attn_w k-tiles
    for k in range(4):
        nc.sync.dma_start(wsb[:, k * A:(k + 1) * A], aw_d[k * 128:(k + 1) * 128, :])
    pwsb = consts.tile([128, 2], DT_R, tag="pwsb")
    absb = consts.tile([128, 2], F32, tag="absb")
    for at in range(2):
        nc.sync.dma_start(pwsb[:, at:at + 1], pw_d[at * 128:(at + 1) * 128, :])
        nc.sync.dma_start(absb[:, at:at + 1], ab_d[at * 128:(at + 1) * 128, :])
    ohsb = consts.tile([BPB, RB], DT_R, tag="ohsb")
    nc.sync.dma_start(ohsb[:], oh_d[:])
    onesb = consts.tile([1, 128], F32, tag="onesb")
    nc.sync.dma_start(onesb[:], on_d[:])
    idsb = consts.tile([128, 128], F32, tag="idsb")
    nc.sync.dma_start(idsb[:], id_d[:])

    x01sb = consts.tile([128, 2 * 2 * BC], F32, tag="x01sb")  # [128,(et,{x0,x1},b)]
    for et in range(2):
        nc.sync.dma_start(x01sb[:, et * 2 * BC:(et + 1) * 2 * BC],
                          x01_d[et * 128:(et + 1) * 128, :])

    epTsb = consts.tile([128, 2 * BC], F32, tag="epTsb")      # [128,(et,b)]
    uTsb = consts.tile([128, 8 * A], DT_R, tag="uTsb")         # [128,(btile,A)]
    rzall = consts.tile([128, 8], F32, tag="rzall")           # 1/(T*Z) per b
    oTacc = consts.tile([128, 2 * BC], F32, tag="oTacc")      # [128,(et,b)]

    # --- prologue: epT, element_product output, uT = ep @ W1 ------------
    for et in range(2):
        nc.vector.tensor_tensor(
            epTsb[:, et * BC:(et + 1) * BC],
            x01sb[:, et * 2 * BC:et * 2 * BC + BC],
            x01sb[:, et * 2 * BC + BC:et * 2 * BC + 2 * BC],
            op=AluOpType.mult,
        )

    with tc.tile_pool(name="prol_ps", bufs=2, space="PSUM") as prol_ps, \
         tc.tile_pool(name="prol_sb", bufs=2) as prol_sb:
        for bt in range(8):
            # element_product natural layout via PE transpose
            epn = prol_sb.tile([128, E], F32, tag="epn")
            for et in range(2):
                pt = prol_ps.tile([128, 128], F32, tag="ept")
                nc.tensor.transpose(pt[:], epTsb[:, et * BC + bt * 128:et * BC + (bt + 1) * 128], idsb[:])
                nc.scalar.copy(epn[:, et * 128:(et + 1) * 128], pt[:])
            nc.sync.dma_start(oe_d[bt * 128:(bt + 1) * 128, :], epn[:])
            # uT tile: ep @ W1  -> [b, A]
            ups = prol_ps.tile([128, A], F32, tag="ups")
            for kt in range(2):
                nc.tensor.matmul(
                    ups[:],
                    epTsb[:, kt * BC + bt * 128:kt * BC + (bt + 1) * 128],
                    _f32(wsb[:, kt * A:(kt + 1) * A]),
                    start=(kt == 0), stop=(kt == 1),
                )
            nc.scalar.copy(uTsb[:, bt * A:(bt + 1) * A], ups[:])

    # --- main pipeline ---------------------------------------------------
    ttp = ctx.enter_context(tc.tile_pool(name="ttp", bufs=2))
    htps = ctx.enter_context(tc.tile_pool(name="htps", bufs=4, space="PSUM"))
    lgps = ctx.enter_context(tc.tile_pool(name="lgps", bufs=2, space="PSUM"))
    sbp = ctx.enter_context(tc.tile_pool(name="sbp", bufs=3))
    smp = ctx.enter_context(tc.tile_pool(name="smp", bufs=2))

    for ch in range(NCHUNK):
        tt0 = ttp.tile([128, CROWS], DT_R, tag="tt0")
        tt1 = ttp.tile([128, CROWS], DT_R, tag="tt1")
        nc.sync.dma_start(tt0[:], tT_d[0:128, ch * CROWS:(ch + 1) * CROWS])
        nc.sync.dma_start(tt1[:], tT_d[128:256, ch * CROWS:(ch + 1) * CROWS])

        # this chunk's uT rows at partition base 0 (PE base-partition rule)
        po_c, col_c = (ch * CB) % 128, (ch * CB) // 128
        uc = smp.tile([BPB, CHUNK_BLKS * A], DT_R, tag="uc")
        for j in range(CHUNK_BLKS):
            po = po_c + j * BPB
            nc.sync.dma_start(uc[:, j * A:(j + 1) * A],
                              uTsb[po:po + BPB, col_c * A:(col_c + 1) * A])

        lgrow = smp.tile([1, CROWS], F32, tag="lgrow")

        for j in range(CHUNK_BLKS):
            g = ch * CHUNK_BLKS + j
            htsb = sbp.tile([128, 2 * RB], DT_R, tag="htsb")
            for at in range(2):
                ht = htps.tile([128, RB], F32, tag="ht")
                nc.tensor.matmul(ht[:], _mmt(wsb[:, 2 * A + at * 128:2 * A + at * 128 + 128]),
                                 _mmt(tt0[:, j * RB:(j + 1) * RB]), start=True, stop=False)
                nc.tensor.matmul(ht[:], _mmt(wsb[:, 3 * A + at * 128:3 * A + at * 128 + 128]),
                                 _mmt(tt1[:, j * RB:(j + 1) * RB]), start=False, stop=False)
                nc.tensor.matmul(ht[:], _mmt(uc[:, j * A + at * 128:j * A + at * 128 + 128]),
                                 _mmt(ohsb[:]), start=False, stop=True)
                nc.scalar.activation(htsb[:, at * RB:(at + 1) * RB], ht[:], AF.Relu,
                                     bias=absb[:, at:at + 1])
            lg = lgps.tile([1, RB], F32, tag="lg")
            nc.tensor.matmul(lg[:], _mmt(pwsb[:, 0:1]), _mmt(htsb[:, 0:RB]),
                             start=True, stop=False)
            nc.tensor.matmul(lg[:], _mmt(pwsb[:, 1:2]), _mmt(htsb[:, RB:2 * RB]),
                             start=False, stop=True)
            nc.scalar.copy(lgrow[:, j * RB:(j + 1) * RB], lg[:])

        # ---- chunk softmax (lane-parallel over 64 batch rows) ----
        lgbt = smp.tile([CB, T], F32, tag="lgbt")
        nc.sync.dma_start(lgbt[:], lgrow[:])
        mx = smp.tile([CB, 1], F32, tag="mx")
        nc.vector.tensor_reduce(mx[:], lgbt[:], axis=AX.X, op=AluOpType.max)
        negmx = smp.tile([CB, 1], F32, tag="negmx")
        nc.vector.tensor_scalar(out=negmx[:], in0=mx[:], scalar1=-1.0, scalar2=None,
                                op0=AluOpType.mult)
        wbt = smp.tile([CB, T], F32, tag="wbt")
        nc.scalar.activation(wbt[:], lgbt[:], AF.Exp, bias=negmx[:])
        zz = smp.tile([CB, 1], F32, tag="zz")
        nc.vector.tensor_reduce(zz[:], wbt[:], axis=AX.X, op=AluOpType.add)
        rz = smp.tile([CB, 1], F32, tag="rz")
        nc.vector.reciprocal(rz[:], zz[:])
        nc.vector.tensor_scalar(out=rzall[po_c:po_c + CB, col_c:col_c + 1], in0=rz[:],
                                scalar1=1.0 / T, scalar2=None, op0=AluOpType.mult)
        wrow = smp.tile([1, CROWS], F32, tag="wrow")
        nc.sync.dma_start(wrow[:], wbt[:])

        # ---- weighted tree sum ----
        for j in range(CHUNK_BLKS):
            g = ch * CHUNK_BLKS + j
            wbc = sbp.tile([128, RB], F32, tag="wbc")
            nc.gpsimd.partition_broadcast(wbc[:], wrow[:, j * RB:(j + 1) * RB])
            m = sbp.tile([128, 2 * RB], F32, tag="m")
            nc.gpsimd.tensor_tensor(m[:, 0:RB], _f32(tt0[:, j * RB:(j + 1) * RB]), wbc[:],
                                    op=AluOpType.mult)
            nc.vector.tensor_tensor(m[:, RB:2 * RB], _f32(tt1[:, j * RB:(j + 1) * RB]), wbc[:],
                                    op=AluOpType.mult)
            for et in range(2):
                nc.vector.tensor_reduce(
                    oTacc[:, et * BC + g * BPB:et * BC + (g + 1) * BPB],
                    m[:, et * RB:(et + 1) * RB].rearrange("p (b t) -> p b t", t=T),
                    axis=AX.X, op=AluOpType.add,
                )

    # --- epilogue: transpose [E, b] -> [b, E], scale by 1/(T*Z) ----------
    with tc.tile_pool(name="epi_ps", bufs=2, space="PSUM") as epi_ps, \
         tc.tile_pool(name="epi_sb", bufs=2) as epi_sb:
        for bt in range(8):
            oasb = epi_sb.tile([128, E], F32, tag="oasb")
            for et in range(2):
                pt = epi_ps.tile([128, 128], F32, tag="opt")
                nc.tensor.transpose(pt[:], oTacc[:, et * BC + bt * 128:et * BC + (bt + 1) * 128], idsb[:])
                nc.vector.tensor_scalar(out=oasb[:, et * 128:(et + 1) * 128], in0=pt[:],
                                        scalar1=rzall[:, bt:bt + 1], scalar2=None,
                                        op0=AluOpType.mult)
            nc.sync.dma_start(oa_d[bt * 128:(bt + 1) * 128, :], oasb[:])


def build():
    if "nc" in _CACHE:
        return _CACHE["nc"]
    nc = bacc.Bacc("TRN2", target_bir_lowering=False, debug=False)
    ins = [
        nc.dram_tensor("treesT", [E, ROWS], DT_R, kind="ExternalInput").ap(),
        nc.dram_tensor("x01T", [E, 2 * BC], F32, kind="ExternalInput").ap(),
        nc.dram_tensor("attn_w", [2 * E, A], DT_R, kind="ExternalInput").ap(),
        nc.dram_tensor("attn_b2", [A, 1], F32, kind="ExternalInput").ap(),
        nc.dram_tensor("proj_w2", [A, 1], DT_R, kind="ExternalInput").ap(),
        nc.dram_tensor("onehot", [BPB, RB], DT_R, kind="ExternalInput").ap(),
        nc.dram_tensor("ones1", [1, 128], F32, kind="ExternalInput").ap(),
        nc.dram_tensor("ident", [128, 128], F32, kind="ExternalInput").ap(),
    ]
    outs = [
        nc.dram_tensor("out_attn", [BC, E], F32, kind="ExternalOutput").ap(),
        nc.dram_tensor("out_ep", [BC, E], F32, kind="ExternalOutput").ap(),
    ]
    with tile.TileContext(nc) as tc, ExitStack() as ctx:
        _body(ctx, tc, ins, outs)
    nc.compile()
    _CACHE["nc"] = nc
    return nc


def make_in_maps(x, attn_w, attn_b, proj_w, proj_b):
    x = np.asarray(x, dtype=np.float32)
    oh = np.zeros((BPB, RB), np.float32)
    for jj in range(BPB):
        oh[jj, jj * T:(jj + 1) * T] = 1.0
    consts = {
        "attn_w": np.ascontiguousarray(np.asarray(attn_w, np.float32)),
        "attn_b2": np.ascontiguousarray(np.asarray(attn_b, np.float32).reshape(A, 1)),
        "proj_w2": np.ascontiguousarray(np.asarray(proj_w, np.float32).reshape(A, 1)),
        "onehot": oh,
        "ones1": np.ones((1, 128), np.float32),
        "ident": np.eye(128, dtype=np.float32),
    }
    in_maps = []
    for c in range(NCORES):
        xs = x[c * BC:(c + 1) * BC]
        treesT = np.ascontiguousarray(xs[:, 2:, :].reshape(ROWS, E).T)
        x01T = np.ascontiguousarray(
            np.concatenate([xs[:, 0, :].T, xs[:, 1, :].T], axis=1))
        in_maps.append({"treesT": treesT, "x01T": x01T, **consts})
    return in_maps


def kernel(x, attn_w, attn_b, proj_w, proj_b):
    global LAST_EXEC_NS, LAST_RESULTS
    nc = build()
    in_maps = make_in_maps(x, attn_w, attn_b, proj_w, proj_b)
    kw = {}
    if PROFILE:
        import shutil
        shutil.rmtree("/tmp/ktrace", ignore_errors=True)
        import os
        os.makedirs("/tmp/ktrace", exist_ok=True)
        kw = dict(trace=True, tmpdir="/tmp/ktrace")
    r = run_bass_kernel_spmd(nc, in_maps, list(range(NCORES)), **kw)
    LAST_EXEC_NS = r.exec_time_ns
    LAST_RESULTS = r
    attn = np.concatenate([r.results[c]["out_attn"] for c in range(NCORES)], axis=0)
    ep = np.concatenate([r.results[c]["out_ep"] for c in range(NCORES)], axis=0)
    return attn, ep
